# revision 5
# baseline (speedup 1.0000x reference)
"""Trainium2 Bass kernel v2 for nn_ADDLossSoftEncode (Davenport q-method ADD loss).

Data parallel over batch: B=512 -> 64 samples/core on 8 cores.
Partition layout: p = 2*s + half (sample-interleaved halves) so every DMA
is a single full-width [128, *] transfer with >=2KB contiguous runs.

Math/schedule vs baseline:
  - softmax max-subtraction dropped (|s|<6 so exp is safe; eigenvectors are
    scale-invariant, the trace identity uses unnormalized sumE).
  - all big elementwise work in bf16 (tolerance 2e-2; measured ~1e-4).
  - q planes 0-2 deinterleaved+cast to bf16 in one multi-plane scalar-engine
    op per chunk; q3 is only ever read by 1x-mode STT ops, which are
    stride-insensitive, so it stays interleaved fp32.
  - Gram pairs: fused product+reduce split across DVE scalar_tensor_tensor,
    gpsimd scalar_tensor_tensor, and DVE-TT product + ACT Identity-accum
    (accums deferred one chunk so the ACT queue never head-of-line blocks).
  - eigen chain (13 trace-normalized squarings) runs on all 128 partitions
    with per-sample data duplicated, so no mid-kernel broadcast DMA sits on
    the critical path.
  - stage C via relative quaternion q_rel = q_pred (x) conj(gt):
    |R_p p - R_g p| = sqrt(4|vp|^2/|v|^2 * (|p|^2 - (vp.p)^2/|vp|^2)),
    vp = L(gt) @ (A^N x0) with L precomputed off the critical path.
  - point DMAs + |p|^2 prep overlap the eigen chain; all DMAs issue from the
    SP queue in compute order.
"""

import sys
from contextlib import ExitStack

import numpy as np

sys.path.insert(0, "/opt/trn_rl_repo")

import concourse.bass as bass
import concourse.tile as tile
from concourse import bacc
from concourse import mybir

F32 = mybir.dt.float32
BF16 = mybir.dt.bfloat16
AX = mybir.AxisListType
OP = mybir.AluOpType
ACT = mybir.ActivationFunctionType

B, K, P = 512, 8192, 4096
NCORES = 8
S = B // NCORES          # 64 samples per core
KH = K // 2              # 4096 per k-half
KC = 1024                # k-chunk width (per half)
NKC = KH // KC           # 4
PH = P // 2              # 2048 points per half
PC = 512                 # point chunk
NPC = PH // PC           # 4

PAIRS = [(0, 0), (0, 1), (0, 2), (0, 3), (1, 1), (1, 2), (1, 3), (2, 2), (2, 3)]
UIDX = {p: n for n, p in enumerate(PAIRS)}
# per-pair engine for even/odd chunks ('act' pairs must have j <= 2):
#   'dve' = DVE STT fused, 'gps' = gpsimd STT fused,
#   'act' = DVE TT product + deferred ACT Identity accumulate
#          pair:  (0,0)   (0,1)   (0,2)  (0,3)   (1,1)   (1,2)   (1,3)   (2,2)   (2,3)
PAIR_ENG = [
    ['dact', 'dact', 'dve', 'pact', 'dact', 'pact', 'dve', 'pact', 'dve'],
    ['dact', 'dve', 'dve', 'pact', 'dact', 'pact', 'pact', 'pact', 'dve'],
    ['dve', 'dact', 'dve', 'pact', 'dact', 'pact', 'dve', 'pact', 'dve'],
    ['dve', 'dact', 'dve', 'pact', 'dve', 'dact', 'dve', 'pact', 'dve'],
]
NSQ = 11                 # matrix squarings
# trace-normalize at these squarings: the first early (unnormalized Gram has
# trace ~ sumE ~ 1.3e4, overflows fp32 by the 4th squaring) and then every 4
# (entries fall to ~0.25^16 between norms; 5+ squarings would hit denormals).
NORM_AT = frozenset({1, 5, 9, NSQ - 1})


def _emit(ctx, tc, sep, ori, gt, pt, out):
    nc = tc.nc
    pool_st = ctx.enter_context(tc.tile_pool(name="st", bufs=1))
    pool_q = ctx.enter_context(tc.tile_pool(name="q", bufs=2))
    pool_qd = ctx.enter_context(tc.tile_pool(name="qd", bufs=2))
    pool_u = ctx.enter_context(tc.tile_pool(name="u", bufs=2))
    pool_pr = ctx.enter_context(tc.tile_pool(name="pr", bufs=3))
    pool_tiny = ctx.enter_context(tc.tile_pool(name="tiny", bufs=1))
    pool_pt = ctx.enter_context(tc.tile_pool(name="pt", bufs=4))
    pool_c = ctx.enter_context(tc.tile_pool(name="c", bufs=2))

    sep_v = sep.rearrange("s (h k) -> (s h) k", h=2)           # [128, 4096]
    ori_v = ori.rearrange("s (h k) q -> (s h) (k q)", h=2)     # [128, 4096*4]
    pt_v = pt.rearrange("s (h p) x -> (s h) (p x)", h=2)       # [128, 2048*3]

    # ---------------- input DMAs (SP queue, compute order) ----------------
    s_sb = pool_st.tile([128, KH], F32)
    e_sb = pool_st.tile([128, KH], BF16)
    gtt = pool_tiny.tile([128, 4], F32)
    # accumulators: pair n chunk c -> col n*NKC+c ; sumE -> cols 9*NKC+c
    aacc = pool_tiny.tile([128, 10 * NKC], F32)

    qcs = []
    for c in range(NKC):
        qc = pool_q.tile([128, KC * 4], F32, tag=f"qc{c % 2}")
        if c == 0:
            half = KC * 2
            nc.sync.dma_start(qc[:, 0:half], ori_v[:, 0:half])
            nc.sync.dma_start(gtt[0:128:2, :], gt[:, :])
            nc.sync.dma_start(gtt[1:128:2, :], gt[:, :])
            nc.sync.dma_start(s_sb[:, 0:KC], sep_v[:, 0:KC])
            nc.sync.dma_start(qc[:, half:KC * 4], ori_v[:, half:KC * 4])
        else:
            nc.sync.dma_start(qc[:], ori_v[:, c * KC * 4:(c + 1) * KC * 4])
            nc.sync.dma_start(s_sb[:, c * KC:(c + 1) * KC],
                              sep_v[:, c * KC:(c + 1) * KC])
        qcs.append(qc)
        if c >= NKC - 2:
            pc_ = c - (NKC - 2)
            ptc = pool_pt.tile([128, PC * 3], F32, tag="ptc")
            nc.sync.dma_start(ptc[:], pt_v[:, pc_ * PC * 3:(pc_ + 1) * PC * 3])
            if c == NKC - 2:
                ptcs = [ptc]
            else:
                ptcs.append(ptc)
    for pc_ in range(2, NPC):
        ptc = pool_pt.tile([128, PC * 3], F32, tag="ptc")
        nc.sync.dma_start(ptc[:], pt_v[:, pc_ * PC * 3:(pc_ + 1) * PC * 3])
        ptcs.append(ptc)

    # ---------------- stage A: exp weights + Gram pairs ----------------
    # accumulations are deferred one chunk so the scalar engine always has
    # ready work and never stalls the chunk pipeline on product completion
    pending_accums = []

    def flush_accums():
        for scr_ap, col_ap in pending_accums:
            nc.scalar.activation(scr_ap, scr_ap, ACT.Identity, accum_out=col_ap)
        pending_accums.clear()

    for c in range(NKC):
        qc = qcs[c]
        ec = e_sb[:, c * KC:(c + 1) * KC]
        nc.scalar.activation(ec, s_sb[:, c * KC:(c + 1) * KC], ACT.Exp,
                             accum_out=aacc[:, 9 * NKC + c:9 * NKC + c + 1])

        # deinterleave q -> 4 bf16 planes (one ACT op; two for chunk 0
        # halves so compute starts as soon as the first half-DMA lands);
        # the deint is qc's only reader, so the staging buffer frees early
        qd = pool_qd.tile([128, 4 * KC], BF16, tag="qd")
        qdv = qd[:].rearrange("p (i k) -> p i k", i=4)
        qcv = qc[:].rearrange("p (k i) -> p i k", i=4)
        if c == 0:
            hk = KC // 2
            nc.scalar.copy(qdv[:, :, 0:hk], qcv[:, :, 0:hk])
            nc.scalar.copy(qdv[:, :, hk:KC], qcv[:, :, hk:KC])
        elif c == 1:
            nc.gpsimd.tensor_copy(qdv, qcv)
        else:
            nc.scalar.copy(qdv, qcv)

        def qdp(i):
            return qd[:, i * KC:(i + 1) * KC]

        qsrc = qdp

        # u_i = e * q_i  (bf16 TT, 2x mode)
        us = []
        for i in range(3):
            ui = pool_u.tile([128, KC], BF16, tag=f"u{i}")
            nc.vector.tensor_tensor(ui[:], ec, qdp(i), op=OP.mult)
            us.append(ui)

        flush_accums()

        pair_eng = PAIR_ENG[c]
        for n, (i, j) in enumerate(PAIRS):
            col = aacc[:, n * NKC + c:n * NKC + c + 1]
            eng = pair_eng[n]
            if eng == 'dve':
                scr = pool_pr.tile([128, KC], BF16, tag="sd")
                nc.vector.scalar_tensor_tensor(
                    scr[:], us[i][:], 1.0, qsrc(j),
                    op0=OP.mult, op1=OP.mult, accum_out=col)
            elif eng == 'pact':  # Pool product + deferred ACT accumulate
                scr = pool_pr.tile([128, KC], BF16, tag=f"sg{n % 2}")
                nc.gpsimd.tensor_tensor(scr[:], us[i][:], qsrc(j), op=OP.mult)
                pending_accums.append((scr[:], col))
            else:  # 'dact': DVE product + fold-by-2 + deferred ACT accumulate
                scr = pool_pr.tile([128, KC], BF16, tag="sa")
                nc.vector.tensor_tensor(scr[:], us[i][:], qsrc(j), op=OP.mult)
                hk = KC // 2
                scr2 = pool_pr.tile([128, hk], BF16, tag="sa2")
                nc.vector.tensor_tensor(scr2[:], scr[:, 0:hk], scr[:, hk:KC],
                                        op=OP.add)
                pending_accums.append((scr2[:], col))

    flush_accums()

    # ---------------- L(gt) build (tiny, off the critical path) -------------
    # vp = vec(v (x) conj(gt)) = L @ v, rows of L [128, 3x4]:
    #   [-gx,  wg, -gz,  gy] ; [-gy,  gz,  wg, -gx] ; [-gz, -gy,  gx,  wg]
    Lm = pool_tiny.tile([128, 12], F32)
    ngt = pool_tiny.tile([128, 4], F32)
    nc.vector.tensor_scalar(ngt[:], gtt[:], -1.0, None, op0=OP.mult)
    lsrc = [(0, 1, True), (1, 0, False), (2, 3, True), (3, 2, False),
            (4, 2, True), (5, 3, False), (6, 0, False), (7, 1, True),
            (8, 3, True), (9, 2, True), (10, 1, False), (11, 0, False)]
    for idx, (dst, src_i, neg) in enumerate(lsrc):
        src = ngt if neg else gtt
        eng = nc.gpsimd if idx % 2 == 0 else nc.vector
        eng.tensor_copy(Lm[:, dst:dst + 1], src[:, src_i:src_i + 1])
    x0t = pool_tiny.tile([128, 4], F32)
    for j, val in enumerate([1.0, 0.61, 0.37, 0.22]):
        nc.gpsimd.memset(x0t[:, j:j + 1], val)
    sqb = pool_tiny.tile([128, 1], F32)
    nc.gpsimd.memset(sqb[:], 0.0)

    # ---------------- stage C prep: planes + |p|^2 (overlaps A/B) -----------
    planes = pool_tiny.tile([128, 3 * PH], BF16)   # x | y | z full planes
    sspl = pool_tiny.tile([128, PH], BF16)
    pn = pool_tiny.tile([128, NPC], F32)
    plv = planes[:].rearrange("p (i k) -> p i k", i=3)
    # pass 1: deinterleave all chunks (Pool for first two, ACT for the rest)
    for c in range(NPC):
        cs = slice(c * PC, (c + 1) * PC)
        if c < 2:
            nc.gpsimd.tensor_copy(plv[:, :, cs],
                                  ptcs[c][:].rearrange("p (k i) -> p i k", i=3))
        else:
            nc.scalar.copy(plv[:, :, cs],
                           ptcs[c][:].rearrange("p (k i) -> p i k", i=3))
    # pass 2: squares and sums per chunk, engine-pass ordered
    t3s = []
    for c in range(NPC):
        x, y, z = (planes[:, i * PH + c * PC:i * PH + (c + 1) * PC]
                   for i in range(3))
        t3 = pool_c.tile([128, PC], BF16, tag=f"t3{c % 2}")
        nc.gpsimd.tensor_tensor(t3[:], z, z, op=OP.mult)
        t3s.append(t3)
    for c in range(NPC):
        cs = slice(c * PC, (c + 1) * PC)
        x, y, z = (planes[:, i * PH + c * PC:i * PH + (c + 1) * PC]
                   for i in range(3))
        t1 = pool_c.tile([128, PC], BF16, tag="t1")
        nc.vector.tensor_tensor(t1[:], x, x, op=OP.mult)
        t2 = pool_c.tile([128, PC], BF16, tag="t2")
        nc.vector.tensor_tensor(t2[:], y, y, op=OP.mult)
        nc.vector.tensor_tensor(t1[:], t1[:], t2[:], op=OP.add)
        nc.vector.tensor_tensor(sspl[:, cs], t1[:], t3s[c][:], op=OP.add)

    # ---------------- Gram combine: halves + chunks -> [128, 10] ------------
    # duplicate the per-sample sums onto both partitions 2s and 2s+1 so the
    # whole eigen chain needs no broadcast afterwards
    NAC = 10 * NKC
    tmp_e = pool_tiny.tile([128, NAC], F32)
    tmp_o = pool_tiny.tile([128, NAC], F32)
    nc.sync.dma_start(tmp_e[0:128:2, :], aacc[0:128:2, :])
    nc.sync.dma_start(tmp_e[1:128:2, :], aacc[0:128:2, :])
    nc.sync.dma_start(tmp_o[0:128:2, :], aacc[1:128:2, :])
    nc.sync.dma_start(tmp_o[1:128:2, :], aacc[1:128:2, :])
    t20 = pool_tiny.tile([128, NAC], F32)
    nc.vector.tensor_tensor(t20[:], tmp_e[:], tmp_o[:], op=OP.add)
    a10 = pool_tiny.tile([128, 10], F32)
    nc.vector.tensor_reduce(a10[:], t20[:].rearrange("p (n c) -> p n c", c=NKC),
                            axis=AX.X, op=OP.add)
    # A33 = sumE - A00 - A11 - A22   (a10 col 9 holds sumE)
    nc.vector.tensor_tensor(a10[:, 9:10], a10[:, 9:10],
                            a10[:, UIDX[(0, 0)]:UIDX[(0, 0)] + 1], op=OP.subtract)
    nc.vector.tensor_tensor(a10[:, 9:10], a10[:, 9:10],
                            a10[:, UIDX[(1, 1)]:UIDX[(1, 1)] + 1], op=OP.subtract)
    nc.vector.tensor_tensor(a10[:, 9:10], a10[:, 9:10],
                            a10[:, UIDX[(2, 2)]:UIDX[(2, 2)] + 1], op=OP.subtract)

    # full 4x4 matrix [128, 16] row-major; slab copies split DVE/gpsimd
    amat = pool_tiny.tile([128, 16], F32)
    copies = [  # (dst_col, src_col, width)
        (0, 0, 4), (4, 1, 1), (5, 4, 3), (8, 2, 1), (9, 5, 1), (10, 7, 2),
        (12, 3, 1), (13, 6, 1), (14, 8, 2),
    ]
    for idx, (dc, sc_, w) in enumerate(copies):
        eng = nc.vector if idx % 2 == 0 else nc.gpsimd
        eng.tensor_copy(amat[:, dc:dc + w], a10[:, sc_:sc_ + w])

    # ---------------- stage B: matrix squaring (A -> A^(2^NSQ)) -------------
    a_cur = amat
    trv = pool_tiny.tile([128, 1], F32)
    tri = pool_tiny.tile([128, 1], F32)
    t64 = pool_tiny.tile([128, 64], F32)
    for m in range(NSQ):
        a_new = pool_tiny.tile([128, 16], F32, tag=f"asq{m % 2}")
        in0 = a_cur[:].rearrange("p (i j) -> p i j", j=4).unsqueeze(2) \
                      .broadcast_to([128, 4, 4, 4])
        in1 = a_cur[:].rearrange("p (j k) -> p k j", k=4).unsqueeze(1) \
                      .broadcast_to([128, 4, 4, 4])
        nc.vector.tensor_tensor(
            t64[:].rearrange("p (i k j) -> p i k j", k=4, j=4), in0, in1,
            op=OP.mult)
        nc.vector.tensor_reduce(
            a_new[:].rearrange("p (i k) -> p i k", k=4),
            t64[:].rearrange("p (ik j) -> p ik j", j=4), axis=AX.X, op=OP.add)
        if m in NORM_AT:
            nc.vector.tensor_reduce(trv[:], a_new[:, 0::5], axis=AX.X, op=OP.add)
            nc.vector.reciprocal(tri[:], trv[:])
            nc.vector.tensor_scalar(a_new[:], a_new[:], tri[:], None, op0=OP.mult)
        a_cur = a_new

    # h = A^N @ x0 ; vp = L @ h ; per-sample scalars (all on 128 partitions)
    t16 = pool_tiny.tile([128, 16], F32)
    h4 = pool_tiny.tile([128, 4], F32)
    nc.vector.tensor_tensor(t16[:].rearrange("p (i j) -> p i j", j=4),
                            a_cur[:].rearrange("p (i j) -> p i j", j=4),
                            x0t[:].unsqueeze(1).broadcast_to([128, 4, 4]),
                            op=OP.mult)
    nc.vector.tensor_reduce(h4[:].unsqueeze(2),
                            t16[:].rearrange("p (i j) -> p i j", j=4),
                            axis=AX.X, op=OP.add)
    scB = pool_tiny.tile([128, 3], F32)   # vpx,vpy,vpz
    scN = pool_tiny.tile([128, 2], F32)   # negr,c1
    t12 = pool_tiny.tile([128, 12], F32)
    nc.vector.tensor_tensor(t12[:].rearrange("p (r j) -> p r j", j=4),
                            Lm[:].rearrange("p (r j) -> p r j", j=4),
                            h4[:].unsqueeze(1).broadcast_to([128, 3, 4]),
                            op=OP.mult)
    nc.vector.tensor_reduce(scB[:, 0:3].unsqueeze(2),
                            t12[:].rearrange("p (r j) -> p r j", j=4),
                            axis=AX.X, op=OP.add)
    # vv2 = |h|^2 on gpsimd, in parallel with the vp chain
    hv2 = pool_tiny.tile([128, 4], F32)
    vv2 = pool_tiny.tile([128, 1], F32)
    nc.vector.scalar_tensor_tensor(hv2[:], h4[:], 1.0, h4[:],
                                   op0=OP.mult, op1=OP.mult, accum_out=vv2[:])
    vp2 = pool_tiny.tile([128, 1], F32)
    sq3 = pool_tiny.tile([128, 3], F32)
    nc.vector.tensor_tensor(sq3[:], scB[:, 0:3], scB[:, 0:3], op=OP.mult)
    nc.vector.tensor_reduce(vp2[:], sq3[:], axis=AX.X, op=OP.add)
    nvp2 = pool_tiny.tile([128, 1], F32)
    nc.vector.tensor_scalar(nvp2[:], vp2[:], -1.0, None, op0=OP.mult)
    nc.vector.reciprocal(scN[:, 0:1], nvp2[:])          # negr = -1/|vp|^2
    c1t = pool_tiny.tile([128, 1], F32)
    nc.vector.reciprocal(c1t[:], vv2[:])
    nc.vector.tensor_tensor(c1t[:], c1t[:], vp2[:], op=OP.mult)
    nc.vector.tensor_scalar(scN[:, 1:2], c1t[:], 4.0, None, op0=OP.mult)

    # ---------------- stage C finish: distances (engine passes) -----------
    ds = []
    for c in range(NPC):
        x, y, z = (planes[:, i * PH + c * PC:i * PH + (c + 1) * PC]
                   for i in range(3))
        d = pool_c.tile([128, PC], BF16, tag=f"d{c % 2}")
        dt = pool_c.tile([128, PC], BF16, tag=f"dt{c % 2}")
        nc.vector.tensor_scalar(dt[:], x, scB[:, 0:1], None, op0=OP.mult)
        nc.vector.scalar_tensor_tensor(d[:], y, scB[:, 1:2], dt[:],
                                       op0=OP.mult, op1=OP.add)
        nc.vector.scalar_tensor_tensor(d[:], z, scB[:, 2:3], d[:],
                                       op0=OP.mult, op1=OP.add)
        ds.append(d)
    dds = []
    for c in range(NPC):
        dd = pool_c.tile([128, PC], BF16, tag=f"dd{c % 2}")
        nc.gpsimd.tensor_tensor(dd[:], ds[c][:], ds[c][:], op=OP.mult)
        dds.append(dd)
    for c in range(NPC):
        cs = slice(c * PC, (c + 1) * PC)
        s2 = pool_c.tile([128, PC], BF16, tag=f"dt{c % 2}")
        nc.vector.scalar_tensor_tensor(s2[:], dds[c][:], scN[:, 0:1],
                                       sspl[:, cs], op0=OP.mult, op1=OP.add)
        nc.vector.tensor_scalar(s2[:], s2[:], 0.0, None, op0=OP.max)
        sq = pool_c.tile([128, PC], BF16, tag=f"t{1 + c % 2}")
        nc.scalar.activation(sq[:], s2[:], ACT.Sqrt, bias=sqb[:],
                             scale=scN[:, 1:2], accum_out=pn[:, c:c + 1])

    partial = pool_tiny.tile([128, 1], F32)
    nc.vector.tensor_reduce(partial[:], pn[:], axis=AX.X, op=OP.add)
    nc.sync.dma_start(out[:, :], partial[:])


_NC_CACHE = {}


def _build():
    if "nc" in _NC_CACHE:
        return _NC_CACHE["nc"]
    nc = bacc.Bacc("TRN2", target_bir_lowering=False, debug=False,
                   enable_asserts=True, num_devices=NCORES)
    sep = nc.declare_dram_parameter("softEncodePred", [S, K], F32, isOutput=False)
    ori = nc.declare_dram_parameter("oriHistogramMap", [S, K, 4], F32, isOutput=False)
    gt = nc.declare_dram_parameter("gt", [S, 4], F32, isOutput=False)
    pt = nc.declare_dram_parameter("point", [S, P, 3], F32, isOutput=False)
    out = nc.declare_dram_parameter("out", [128, 1], F32, isOutput=True)
    with tile.TileContext(nc) as tc:
        with ExitStack() as ctx:
            _emit(ctx, tc, sep.ap(), ori.ap(), gt.ap(), pt.ap(), out.ap())
    nc.finalize()
    _NC_CACHE["nc"] = nc
    return nc


def kernel(softEncodePred, oriHistogramMap, gt, point):
    nc = _build()
    in_maps = []
    for c in range(NCORES):
        sl = slice(c * S, (c + 1) * S)
        in_maps.append({
            "softEncodePred": np.ascontiguousarray(softEncodePred[sl], np.float32),
            "oriHistogramMap": np.ascontiguousarray(oriHistogramMap[sl], np.float32),
            "gt": np.ascontiguousarray(gt[sl], np.float32),
            "point": np.ascontiguousarray(point[sl], np.float32),
        })
    from concourse.bass_utils import run_bass_kernel_spmd
    res = run_bass_kernel_spmd(nc, in_maps, core_ids=list(range(NCORES)))
    total = np.float64(0.0)
    for r in res.results:
        total += np.asarray(r["out"], np.float64).sum()
    return np.float32(total / (B * P))


# revision 7
# speedup vs baseline: 1.0689x; 1.0689x over previous
"""Trainium2 Bass kernel v2 for nn_ADDLossSoftEncode (Davenport q-method ADD loss).

Data parallel over batch: B=512 -> 64 samples/core on 8 cores.
Partition layout: p = 2*s + half (sample-interleaved halves) so every DMA
is a single full-width [128, *] transfer with >=2KB contiguous runs.

Math/schedule vs baseline:
  - softmax max-subtraction dropped (|s|<6 so exp is safe; eigenvectors are
    scale-invariant, the trace identity uses unnormalized sumE).
  - all big elementwise work in bf16 (tolerance 2e-2; measured ~1e-4).
  - q planes 0-2 deinterleaved+cast to bf16 in one multi-plane scalar-engine
    op per chunk; q3 is only ever read by 1x-mode STT ops, which are
    stride-insensitive, so it stays interleaved fp32.
  - Gram pairs: fused product+reduce split across DVE scalar_tensor_tensor,
    gpsimd scalar_tensor_tensor, and DVE-TT product + ACT Identity-accum
    (accums deferred one chunk so the ACT queue never head-of-line blocks).
  - eigen chain (13 trace-normalized squarings) runs on all 128 partitions
    with per-sample data duplicated, so no mid-kernel broadcast DMA sits on
    the critical path.
  - stage C via relative quaternion q_rel = q_pred (x) conj(gt):
    |R_p p - R_g p| = sqrt(4|vp|^2/|v|^2 * (|p|^2 - (vp.p)^2/|vp|^2)),
    vp = L(gt) @ (A^N x0) with L precomputed off the critical path.
  - point DMAs + |p|^2 prep overlap the eigen chain; all DMAs issue from the
    SP queue in compute order.
"""

import sys
from contextlib import ExitStack

import numpy as np

sys.path.insert(0, "/opt/trn_rl_repo")

import concourse.bass as bass
import concourse.tile as tile
from concourse import bacc
from concourse import mybir

F32 = mybir.dt.float32
BF16 = mybir.dt.bfloat16
AX = mybir.AxisListType
OP = mybir.AluOpType
ACT = mybir.ActivationFunctionType

B, K, P = 512, 8192, 4096
NCORES = 8
S = B // NCORES          # 64 samples per core
KH = K // 2              # 4096 per k-half
KC = 1024                # k-chunk width (per half)
NKC = KH // KC           # 4
PH = P // 2              # 2048 points per half
PC = 512                 # point chunk
NPC = PH // PC           # 4

PAIRS = [(0, 0), (0, 1), (0, 2), (0, 3), (1, 1), (1, 2), (1, 3), (2, 2), (2, 3)]
UIDX = {p: n for n, p in enumerate(PAIRS)}
# per-pair engine for even/odd chunks ('act' pairs must have j <= 2):
#   'dve' = DVE STT fused, 'gps' = gpsimd STT fused,
#   'act' = DVE TT product + deferred ACT Identity accumulate
#          pair:  (0,0)   (0,1)   (0,2)  (0,3)   (1,1)   (1,2)   (1,3)   (2,2)   (2,3)
PAIR_ENG = [
    ['dact', 'dact', 'dve', 'pact', 'dact', 'pact', 'dve', 'pact', 'dve'],
    ['dact', 'dve', 'dve', 'pact', 'dact', 'pact', 'pact', 'pact', 'dve'],
    ['dve', 'dact', 'dve', 'pact', 'dact', 'pact', 'dve', 'pact', 'dve'],
    ['dve', 'dact', 'dve', 'pact', 'dact', 'dact', 'pact', 'pact', 'dve'],
]
NSQ = 11                 # matrix squarings
# trace-normalize at these squarings: the first early (unnormalized Gram has
# trace ~ sumE ~ 1.3e4, overflows fp32 by the 4th squaring) and then every 4
# (entries fall to ~0.25^16 between norms; 5+ squarings would hit denormals).
NORM_AT = frozenset({1, 5, 9})  # no final norm: vp scale cancels downstream


def _emit(ctx, tc, sep, ori, gt, pt, out):
    nc = tc.nc
    pool_st = ctx.enter_context(tc.tile_pool(name="st", bufs=1))
    pool_q = ctx.enter_context(tc.tile_pool(name="q", bufs=2))
    pool_qd = ctx.enter_context(tc.tile_pool(name="qd", bufs=2))
    pool_u = ctx.enter_context(tc.tile_pool(name="u", bufs=2))
    pool_pr = ctx.enter_context(tc.tile_pool(name="pr", bufs=3))
    pool_tiny = ctx.enter_context(tc.tile_pool(name="tiny", bufs=1))
    pool_pt = ctx.enter_context(tc.tile_pool(name="pt", bufs=4))
    pool_c = ctx.enter_context(tc.tile_pool(name="c", bufs=2))

    sep_v = sep.rearrange("s (h k) -> (s h) k", h=2)           # [128, 4096]
    ori_v = ori.rearrange("s (h k) q -> (s h) (k q)", h=2)     # [128, 4096*4]
    pt_v = pt.rearrange("s (h p) x -> (s h) (p x)", h=2)       # [128, 2048*3]

    # ---------------- input DMAs (SP queue, compute order) ----------------
    s_sb = pool_st.tile([128, KH], F32)
    e_sb = pool_st.tile([128, KH], BF16)
    gtt = pool_tiny.tile([128, 4], F32)
    # accumulators: pair n chunk c -> col n*NKC+c ; sumE -> cols 9*NKC+c
    aacc = pool_tiny.tile([128, 10 * NKC], F32)

    qcs = []
    for c in range(NKC):
        qc = pool_q.tile([128, KC * 4], F32, tag=f"qc{c % 2}")
        if c == 0:
            half = KC * 2
            nc.sync.dma_start(qc[:, 0:half], ori_v[:, 0:half])
            nc.sync.dma_start(gtt[0:128:2, :], gt[:, :])
            nc.sync.dma_start(gtt[1:128:2, :], gt[:, :])
            nc.sync.dma_start(s_sb[:, 0:KC], sep_v[:, 0:KC])
            nc.sync.dma_start(qc[:, half:KC * 4], ori_v[:, half:KC * 4])
        else:
            nc.sync.dma_start(qc[:], ori_v[:, c * KC * 4:(c + 1) * KC * 4])
            nc.sync.dma_start(s_sb[:, c * KC:(c + 1) * KC],
                              sep_v[:, c * KC:(c + 1) * KC])
        qcs.append(qc)
        if c >= NKC - 2:
            pc_ = c - (NKC - 2)
            ptc = pool_pt.tile([128, PC * 3], F32, tag="ptc")
            nc.sync.dma_start(ptc[:], pt_v[:, pc_ * PC * 3:(pc_ + 1) * PC * 3])
            if c == NKC - 2:
                ptcs = [ptc]
            else:
                ptcs.append(ptc)
    for pc_ in range(2, NPC):
        ptc = pool_pt.tile([128, PC * 3], F32, tag="ptc")
        nc.sync.dma_start(ptc[:], pt_v[:, pc_ * PC * 3:(pc_ + 1) * PC * 3])
        ptcs.append(ptc)

    # ---------------- stage A: exp weights + Gram pairs ----------------
    # accumulations are deferred one chunk so the scalar engine always has
    # ready work and never stalls the chunk pipeline on product completion
    pending_accums = []

    def flush_accums():
        for scr_ap, col_ap in pending_accums:
            nc.scalar.activation(scr_ap, scr_ap, ACT.Identity, accum_out=col_ap)
        pending_accums.clear()

    for c in range(NKC):
        qc = qcs[c]
        ec = e_sb[:, c * KC:(c + 1) * KC]
        nc.scalar.activation(ec, s_sb[:, c * KC:(c + 1) * KC], ACT.Exp,
                             accum_out=aacc[:, 9 * NKC + c:9 * NKC + c + 1])

        # deinterleave q -> 4 bf16 planes (one ACT op; two for chunk 0
        # halves so compute starts as soon as the first half-DMA lands);
        # the deint is qc's only reader, so the staging buffer frees early
        qd = pool_qd.tile([128, 4 * KC], BF16, tag="qd")
        qdv = qd[:].rearrange("p (i k) -> p i k", i=4)
        qcv = qc[:].rearrange("p (k i) -> p i k", i=4)
        if c == 0:
            hk = KC // 2
            nc.scalar.copy(qdv[:, :, 0:hk], qcv[:, :, 0:hk])
            nc.scalar.copy(qdv[:, :, hk:KC], qcv[:, :, hk:KC])
        else:
            nc.scalar.copy(qdv, qcv)

        def qdp(i):
            return qd[:, i * KC:(i + 1) * KC]

        qsrc = qdp

        # u_i = e * q_i  (bf16 TT, 2x mode)
        us = []
        for i in range(3):
            ui = pool_u.tile([128, KC], BF16, tag=f"u{i}")
            nc.vector.tensor_tensor(ui[:], ec, qdp(i), op=OP.mult)
            us.append(ui)

        flush_accums()

        pair_eng = PAIR_ENG[c]
        for n, (i, j) in enumerate(PAIRS):
            col = aacc[:, n * NKC + c:n * NKC + c + 1]
            eng = pair_eng[n]
            if eng == 'dve':
                scr = pool_pr.tile([128, KC], BF16, tag="sd")
                nc.vector.scalar_tensor_tensor(
                    scr[:], us[i][:], 1.0, qsrc(j),
                    op0=OP.mult, op1=OP.mult, accum_out=col)
            elif eng == 'pact':  # Pool product + deferred ACT accumulate
                scr = pool_pr.tile([128, KC], BF16, tag=f"sg{n % 2}")
                nc.gpsimd.tensor_tensor(scr[:], us[i][:], qsrc(j), op=OP.mult)
                pending_accums.append((scr[:], col))
            else:  # 'dact': DVE product + fold-by-2 + deferred ACT accumulate
                scr = pool_pr.tile([128, KC], BF16, tag="sa")
                nc.vector.tensor_tensor(scr[:], us[i][:], qsrc(j), op=OP.mult)
                hk = KC // 2
                scr2 = pool_pr.tile([128, hk], BF16, tag="sa2")
                nc.vector.tensor_tensor(scr2[:], scr[:, 0:hk], scr[:, hk:KC],
                                        op=OP.add)
                pending_accums.append((scr2[:], col))

    flush_accums()

    # ---------------- L(gt) build (tiny, off the critical path) -------------
    # vp = vec(v (x) conj(gt)) = L @ v, rows of L [128, 3x4]:
    #   [-gx,  wg, -gz,  gy] ; [-gy,  gz,  wg, -gx] ; [-gz, -gy,  gx,  wg]
    Lm = pool_tiny.tile([128, 12], F32)
    ngt = pool_tiny.tile([128, 4], F32)
    nc.vector.tensor_scalar(ngt[:], gtt[:], -1.0, None, op0=OP.mult)
    lsrc = [(0, 1, True), (1, 0, False), (2, 3, True), (3, 2, False),
            (4, 2, True), (5, 3, False), (6, 0, False), (7, 1, True),
            (8, 3, True), (9, 2, True), (10, 1, False), (11, 0, False)]
    for idx, (dst, src_i, neg) in enumerate(lsrc):
        src = ngt if neg else gtt
        eng = nc.gpsimd if idx % 2 == 0 else nc.vector
        eng.tensor_copy(Lm[:, dst:dst + 1], src[:, src_i:src_i + 1])
    x0t = pool_tiny.tile([128, 4], F32)
    for j, val in enumerate([1.0, 0.61, 0.37, 0.22]):
        nc.gpsimd.memset(x0t[:, j:j + 1], val)
    sqb = pool_tiny.tile([128, 1], F32)
    nc.gpsimd.memset(sqb[:], 0.0)

    # ---------------- stage C prep: planes + |p|^2 (overlaps A/B) -----------
    planes = pool_tiny.tile([128, 3 * PH], BF16)   # x | y | z full planes
    sspl = pool_tiny.tile([128, PH], BF16)
    pn = pool_tiny.tile([128, NPC], F32)
    plv = planes[:].rearrange("p (i k) -> p i k", i=3)
    # pass 1: deinterleave all chunks (Pool for first two, ACT for the rest)
    for c in range(NPC):
        cs = slice(c * PC, (c + 1) * PC)
        if c < 2:
            nc.gpsimd.tensor_copy(plv[:, :, cs],
                                  ptcs[c][:].rearrange("p (k i) -> p i k", i=3))
        else:
            nc.scalar.copy(plv[:, :, cs],
                           ptcs[c][:].rearrange("p (k i) -> p i k", i=3))
    # pass 2: squares and sums per chunk, engine-pass ordered
    t3s = []
    for c in range(NPC):
        x, y, z = (planes[:, i * PH + c * PC:i * PH + (c + 1) * PC]
                   for i in range(3))
        t3 = pool_c.tile([128, PC], BF16, tag=f"t3{c % 2}")
        nc.gpsimd.tensor_tensor(t3[:], z, z, op=OP.mult)
        t3s.append(t3)
    for c in range(NPC):
        cs = slice(c * PC, (c + 1) * PC)
        x, y, z = (planes[:, i * PH + c * PC:i * PH + (c + 1) * PC]
                   for i in range(3))
        t1 = pool_c.tile([128, PC], BF16, tag="t1")
        nc.vector.tensor_tensor(t1[:], x, x, op=OP.mult)
        t2 = pool_c.tile([128, PC], BF16, tag="t2")
        nc.vector.tensor_tensor(t2[:], y, y, op=OP.mult)
        nc.vector.tensor_tensor(t1[:], t1[:], t2[:], op=OP.add)
        nc.vector.tensor_tensor(sspl[:, cs], t1[:], t3s[c][:], op=OP.add)

    # ---------------- Gram combine: halves + chunks -> [128, 10] ------------
    # duplicate the per-sample sums onto both partitions 2s and 2s+1 so the
    # whole eigen chain needs no broadcast afterwards
    NAC = 10 * NKC
    tmp_e = pool_tiny.tile([128, NAC], F32)
    tmp_o = pool_tiny.tile([128, NAC], F32)
    nc.sync.dma_start(tmp_e[0:128:2, :], aacc[0:128:2, :])
    nc.sync.dma_start(tmp_e[1:128:2, :], aacc[0:128:2, :])
    nc.sync.dma_start(tmp_o[0:128:2, :], aacc[1:128:2, :])
    nc.sync.dma_start(tmp_o[1:128:2, :], aacc[1:128:2, :])
    t20 = pool_tiny.tile([128, NAC], F32)
    nc.vector.tensor_tensor(t20[:], tmp_e[:], tmp_o[:], op=OP.add)
    a10 = pool_tiny.tile([128, 10], F32)
    nc.vector.tensor_reduce(a10[:], t20[:].rearrange("p (n c) -> p n c", c=NKC),
                            axis=AX.X, op=OP.add)
    # A33 = sumE - A00 - A11 - A22   (a10 col 9 holds sumE)
    nc.vector.tensor_tensor(a10[:, 9:10], a10[:, 9:10],
                            a10[:, UIDX[(0, 0)]:UIDX[(0, 0)] + 1], op=OP.subtract)
    nc.vector.tensor_tensor(a10[:, 9:10], a10[:, 9:10],
                            a10[:, UIDX[(1, 1)]:UIDX[(1, 1)] + 1], op=OP.subtract)
    nc.vector.tensor_tensor(a10[:, 9:10], a10[:, 9:10],
                            a10[:, UIDX[(2, 2)]:UIDX[(2, 2)] + 1], op=OP.subtract)

    # full 4x4 matrix [128, 16] row-major; slab copies split DVE/gpsimd
    amat = pool_tiny.tile([128, 16], F32)
    copies = [  # (dst_col, src_col, width)
        (0, 0, 4), (4, 1, 1), (5, 4, 3), (8, 2, 1), (9, 5, 1), (10, 7, 2),
        (12, 3, 1), (13, 6, 1), (14, 8, 2),
    ]
    for idx, (dc, sc_, w) in enumerate(copies):
        eng = nc.vector if idx % 2 == 0 else nc.gpsimd
        eng.tensor_copy(amat[:, dc:dc + w], a10[:, sc_:sc_ + w])

    # ---------------- stage B: matrix squaring (A -> A^(2^NSQ)) -------------
    a_cur = amat
    trv = pool_tiny.tile([128, 1], F32)
    tri = pool_tiny.tile([128, 1], F32)
    t64 = pool_tiny.tile([128, 64], F32)
    for m in range(NSQ):
        a_new = pool_tiny.tile([128, 16], F32, tag=f"asq{m % 2}")
        in0 = a_cur[:].rearrange("p (i j) -> p i j", j=4).unsqueeze(2) \
                      .broadcast_to([128, 4, 4, 4])
        in1 = a_cur[:].rearrange("p (j k) -> p k j", k=4).unsqueeze(1) \
                      .broadcast_to([128, 4, 4, 4])
        nc.vector.tensor_tensor(
            t64[:].rearrange("p (i k j) -> p i k j", k=4, j=4), in0, in1,
            op=OP.mult)
        nc.vector.tensor_reduce(
            a_new[:].rearrange("p (i k) -> p i k", k=4),
            t64[:].rearrange("p (ik j) -> p ik j", j=4), axis=AX.X, op=OP.add)
        if m in NORM_AT:
            nc.vector.tensor_reduce(trv[:], a_new[:, 0::5], axis=AX.X, op=OP.add)
            nc.vector.reciprocal(tri[:], trv[:])
            nc.vector.tensor_scalar(a_new[:], a_new[:], tri[:], None, op0=OP.mult)
        a_cur = a_new

    # h = A^N @ x0 ; vp = L @ h ; per-sample scalars (all on 128 partitions)
    t16 = pool_tiny.tile([128, 16], F32)
    h4 = pool_tiny.tile([128, 4], F32)
    nc.vector.tensor_tensor(t16[:].rearrange("p (i j) -> p i j", j=4),
                            a_cur[:].rearrange("p (i j) -> p i j", j=4),
                            x0t[:].unsqueeze(1).broadcast_to([128, 4, 4]),
                            op=OP.mult)
    nc.vector.tensor_reduce(h4[:].unsqueeze(2),
                            t16[:].rearrange("p (i j) -> p i j", j=4),
                            axis=AX.X, op=OP.add)
    scB = pool_tiny.tile([128, 3], F32)   # vpx,vpy,vpz
    scN = pool_tiny.tile([128, 2], F32)   # negr,c1
    t12 = pool_tiny.tile([128, 12], F32)
    nc.vector.tensor_tensor(t12[:].rearrange("p (r j) -> p r j", j=4),
                            Lm[:].rearrange("p (r j) -> p r j", j=4),
                            h4[:].unsqueeze(1).broadcast_to([128, 3, 4]),
                            op=OP.mult)
    nc.vector.tensor_reduce(scB[:, 0:3].unsqueeze(2),
                            t12[:].rearrange("p (r j) -> p r j", j=4),
                            axis=AX.X, op=OP.add)
    # vv2 = |h|^2 on gpsimd, in parallel with the vp chain
    hv2 = pool_tiny.tile([128, 4], F32)
    vv2 = pool_tiny.tile([128, 1], F32)
    nc.vector.scalar_tensor_tensor(hv2[:], h4[:], 1.0, h4[:],
                                   op0=OP.mult, op1=OP.mult, accum_out=vv2[:])
    vp2 = pool_tiny.tile([128, 1], F32)
    sq3 = pool_tiny.tile([128, 3], F32)
    nc.vector.tensor_tensor(sq3[:], scB[:, 0:3], scB[:, 0:3], op=OP.mult)
    nc.vector.tensor_reduce(vp2[:], sq3[:], axis=AX.X, op=OP.add)
    nvp2 = pool_tiny.tile([128, 1], F32)
    nc.vector.tensor_scalar(nvp2[:], vp2[:], -1.0, None, op0=OP.mult)
    nc.vector.reciprocal(scN[:, 0:1], nvp2[:])          # negr = -1/|vp|^2
    c1t = pool_tiny.tile([128, 1], F32)
    nc.vector.reciprocal(c1t[:], vv2[:])
    nc.vector.tensor_tensor(c1t[:], c1t[:], vp2[:], op=OP.mult)
    nc.vector.tensor_scalar(scN[:, 1:2], c1t[:], 4.0, None, op0=OP.mult)

    # ---------------- stage C finish: distances (engine passes) -----------
    ds = []
    for c in range(NPC):
        x, y, z = (planes[:, i * PH + c * PC:i * PH + (c + 1) * PC]
                   for i in range(3))
        d = pool_c.tile([128, PC], BF16, tag=f"d{c % 2}")
        dt = pool_c.tile([128, PC], BF16, tag=f"dt{c % 2}")
        nc.vector.tensor_scalar(dt[:], x, scB[:, 0:1], None, op0=OP.mult)
        nc.vector.scalar_tensor_tensor(d[:], y, scB[:, 1:2], dt[:],
                                       op0=OP.mult, op1=OP.add)
        nc.vector.scalar_tensor_tensor(d[:], z, scB[:, 2:3], d[:],
                                       op0=OP.mult, op1=OP.add)
        ds.append(d)
    dds = []
    for c in range(NPC):
        dd = pool_c.tile([128, PC], BF16, tag=f"dd{c % 2}")
        nc.gpsimd.tensor_tensor(dd[:], ds[c][:], ds[c][:], op=OP.mult)
        dds.append(dd)
    for c in range(NPC):
        cs = slice(c * PC, (c + 1) * PC)
        s2 = pool_c.tile([128, PC], BF16, tag=f"dt{c % 2}")
        nc.vector.scalar_tensor_tensor(s2[:], dds[c][:], scN[:, 0:1],
                                       sspl[:, cs], op0=OP.mult, op1=OP.add)
        nc.vector.tensor_scalar(s2[:], s2[:], 0.0, None, op0=OP.max)
        sq = pool_c.tile([128, PC], BF16, tag=f"t{1 + c % 2}")
        nc.scalar.activation(sq[:], s2[:], ACT.Sqrt, bias=sqb[:],
                             scale=scN[:, 1:2], accum_out=pn[:, c:c + 1])

    partial = pool_tiny.tile([128, 1], F32)
    nc.vector.tensor_reduce(partial[:], pn[:], axis=AX.X, op=OP.add)
    nc.sync.dma_start(out[:, :], partial[:])


_NC_CACHE = {}


def _build():
    if "nc" in _NC_CACHE:
        return _NC_CACHE["nc"]
    nc = bacc.Bacc("TRN2", target_bir_lowering=False, debug=False,
                   enable_asserts=True, num_devices=NCORES)
    sep = nc.declare_dram_parameter("softEncodePred", [S, K], F32, isOutput=False)
    ori = nc.declare_dram_parameter("oriHistogramMap", [S, K, 4], F32, isOutput=False)
    gt = nc.declare_dram_parameter("gt", [S, 4], F32, isOutput=False)
    pt = nc.declare_dram_parameter("point", [S, P, 3], F32, isOutput=False)
    out = nc.declare_dram_parameter("out", [128, 1], F32, isOutput=True)
    with tile.TileContext(nc) as tc:
        with ExitStack() as ctx:
            _emit(ctx, tc, sep.ap(), ori.ap(), gt.ap(), pt.ap(), out.ap())
    nc.finalize()
    _NC_CACHE["nc"] = nc
    return nc


def kernel(softEncodePred, oriHistogramMap, gt, point):
    nc = _build()
    in_maps = []
    for c in range(NCORES):
        sl = slice(c * S, (c + 1) * S)
        in_maps.append({
            "softEncodePred": np.ascontiguousarray(softEncodePred[sl], np.float32),
            "oriHistogramMap": np.ascontiguousarray(oriHistogramMap[sl], np.float32),
            "gt": np.ascontiguousarray(gt[sl], np.float32),
            "point": np.ascontiguousarray(point[sl], np.float32),
        })
    from concourse.bass_utils import run_bass_kernel_spmd
    res = run_bass_kernel_spmd(nc, in_maps, core_ids=list(range(NCORES)))
    total = np.float64(0.0)
    for r in res.results:
        total += np.asarray(r["out"], np.float64).sum()
    return np.float32(total / (B * P))
